# revision 2
# baseline (speedup 1.0000x reference)
"""MoE gated-sum kernel for Trainium2 (8 NeuronCores, batch-sharded).

Problem: out[b,c,h,w] = sum_e l_learner[e,b,c,h,w] * g[b, e*512 + c]
  l_learner: [8, 8, 512, 56, 56] f32, g: [8, 4096] f32 -> out [8, 512, 56, 56] f32

Sharding: batch-parallel over the 8 cores (B == n_cores). Each core gets
l_learner[:, b] (contiguous copy, 51.4 MB) plus the per-batch gates
transposed to [C, E], computes its full [512, 56*56] output slice, and the
host stacks the slices. No collectives needed (unlike expert-parallel,
which would all-reduce 51.4 MB partials per core).

Per-core program (raw Bass, explicit semaphores): for each of 4 channel
tiles (128 partitions x 3136 free) accumulate the 8 experts on the vector
engine:
  e=0: acc = l_0 * g[:,0]         (tensor_scalar, 2x perf mode for f32)
  e>0: acc = (l_e * g[:,e]) + acc (fused scalar_tensor_tensor MAC)
Loads stream on the sync-engine HWDGE ring (fully contiguous 1.6 MB
blocks, NBUF-deep pipeline), stores go out on the scalar-engine HWDGE
ring so they never block the load stream.
"""

import numpy as np

import concourse.bass as bass
import concourse.mybir as mybir
from concourse.bass_utils import run_bass_kernel_spmd

N_EXPERTS = 8
BATCH = 8
CHANNELS = 512
H = W = 56
S = H * W  # 3136
N_CORES = 8
P = 128
N_CTILES = CHANNELS // P  # 4
NBUF = 6  # l-tile ring depth (6 x 12.5KB/partition)

_FP32 = mybir.dt.float32
_program = None


def _build_program() -> bass.Bass:
    E, C = N_EXPERTS, CHANNELS
    nc = bass.Bass()
    l = nc.declare_dram_parameter("l", [E, C, S], _FP32, isOutput=False)
    gt = nc.declare_dram_parameter("gt", [C, E], _FP32, isOutput=False)
    out = nc.declare_dram_parameter("out", [C, S], _FP32, isOutput=True)

    n_ops = N_CTILES * E  # 32 expert-accumulate steps

    with (
        nc.sbuf_tensor([P, NBUF * S], _FP32) as lbuf,
        nc.sbuf_tensor([P, 2 * S], _FP32) as accbuf,
        nc.sbuf_tensor([P, N_CTILES * E], _FP32) as gbuf,
        nc.semaphore("ld_sem") as ld_sem,  # load completions (sync HWDGE ring)
        nc.semaphore("st_sem") as st_sem,  # store completions (scalar HWDGE ring)
        nc.semaphore("v_sem") as v_sem,  # vector op completions
        nc.Block() as block,
    ):

        @block.sync
        def _(sync):
            for ci in range(N_CTILES):
                sync.dma_start(
                    out=gbuf[:, ci * E : (ci + 1) * E],
                    in_=gt[ci * P : (ci + 1) * P, :],
                ).then_inc(ld_sem, 16)
            for idx in range(n_ops):
                ci, e = divmod(idx, E)
                slot = idx % NBUF
                if idx >= NBUF:
                    # slot reused: its previous occupant must be consumed
                    sync.wait_ge(v_sem, idx - NBUF + 1)
                sync.dma_start(
                    out=lbuf[:, slot * S : (slot + 1) * S],
                    in_=l[e, ci * P : (ci + 1) * P, :],
                ).then_inc(ld_sem, 16)

        @block.vector
        def _(vector):
            for idx in range(n_ops):
                ci, e = divmod(idx, E)
                slot = idx % NBUF
                acc = accbuf[:, (ci % 2) * S : (ci % 2 + 1) * S]
                lt = lbuf[:, slot * S : (slot + 1) * S]
                gcol = gbuf[:, ci * E + e : ci * E + e + 1]
                vector.wait_ge(ld_sem, 16 * (N_CTILES + idx + 1))
                if e == 0:
                    if ci >= 2:
                        # acc slot recycled: store of ci-2 must be done
                        vector.wait_ge(st_sem, 16 * (ci - 1))
                    vector.tensor_scalar_mul(acc, lt, gcol).then_inc(v_sem, 1)
                else:
                    vector.scalar_tensor_tensor(
                        acc,
                        lt,
                        gcol,
                        acc,
                        op0=mybir.AluOpType.mult,
                        op1=mybir.AluOpType.add,
                    ).then_inc(v_sem, 1)

        @block.scalar
        def _(scalar):
            for ci in range(N_CTILES):
                scalar.wait_ge(v_sem, E * (ci + 1))
                scalar.dma_start(
                    out=out[ci * P : (ci + 1) * P, :],
                    in_=accbuf[:, (ci % 2) * S : (ci % 2 + 1) * S],
                ).then_inc(st_sem, 16)
            scalar.wait_ge(st_sem, 16 * N_CTILES)

    return nc


def _get_program() -> bass.Bass:
    global _program
    if _program is None:
        _program = _build_program()
    return _program


def _shard_inputs(l_learner: np.ndarray, g: np.ndarray) -> list[dict[str, np.ndarray]]:
    l_learner = np.asarray(l_learner, dtype=np.float32)
    g = np.asarray(g, dtype=np.float32)
    in_maps = []
    for b in range(BATCH):
        lb = np.ascontiguousarray(l_learner[:, b]).reshape(N_EXPERTS, CHANNELS, S)
        gb = np.ascontiguousarray(g[b].reshape(N_EXPERTS, CHANNELS).T)
        in_maps.append({"l": lb, "gt": gb})
    return in_maps


def kernel(l_learner: np.ndarray, g: np.ndarray) -> np.ndarray:
    nc = _get_program()
    in_maps = _shard_inputs(l_learner, g)
    res = run_bass_kernel_spmd(nc, in_maps, list(range(N_CORES)))
    return np.stack(
        [res.results[b]["out"].reshape(CHANNELS, H, W) for b in range(BATCH)], axis=0
    )


# revision 4
# speedup vs baseline: 246.1631x; 246.1631x over previous
"""MoE gated-sum kernel for Trainium2 (8 NeuronCores, batch-sharded).

Problem: out[b,c,h,w] = sum_e l_learner[e,b,c,h,w] * g[b, e*512 + c]
  l_learner: [8, 8, 512, 56, 56] f32, g: [8, 4096] f32 -> out [8, 512, 56, 56] f32

Sharding: batch-parallel over the 8 cores (B == n_cores). Each core gets
l_learner[:, b] (contiguous copy, 51.4 MB) plus the per-batch gates
transposed to [C, E], computes its full [512, 56*56] output slice, and the
host stacks the slices. No collectives needed (unlike expert-parallel,
which would all-reduce 51.4 MB partials per core).

Per-core program (raw Bass, explicit semaphores): for each of 4 channel
tiles (128 partitions x 3136 free) accumulate the 8 experts on the vector
engine:
  e=0: acc = l_0 * g[:,0]         (tensor_scalar, 2x perf mode for f32)
  e>0: acc = (l_e * g[:,e]) + acc (fused scalar_tensor_tensor MAC)
Loads stream on the sync-engine HWDGE ring (fully contiguous 1.6 MB
blocks, NBUF-deep pipeline), stores go out on the scalar-engine HWDGE
ring so they never block the load stream.
"""

import numpy as np

import concourse.bass as bass
import concourse.mybir as mybir
from concourse.bass_utils import run_bass_kernel_spmd

N_EXPERTS = 8
BATCH = 8
CHANNELS = 512
H = W = 56
S = H * W  # 3136
N_CORES = 8
P = 128
N_CTILES = CHANNELS // P  # 4
NBUF = 6  # l-tile ring depth (6 x 12.5KB/partition)

_FP32 = mybir.dt.float32
_program = None


def _build_program(reps: int = 1) -> bass.Bass:
    """Build the per-core program. ``reps`` repeats the whole body (same
    result, re-stored each rep) — used only for slope-based wall-clock
    timing in test.py, since this container has no NTFF profiling.

    Semaphore discipline: sem increments from concurrently-outstanding DMAs
    on one counting semaphore can interleave (the 16 per-SDMA-engine incs
    of DMA i+1 can land before DMA i's are all in), so a cumulative
    wait_ge(sem, 16*i) does NOT prove DMA i finished. Every data-carrying
    DMA therefore gets a semaphore on which at most ONE transfer is ever
    outstanding: one sem per l-tile ring slot, one per acc parity. The
    pipeline dependencies themselves enforce the one-outstanding rule."""
    E, C = N_EXPERTS, CHANNELS
    nc = bass.Bass()
    l = nc.declare_dram_parameter("l", [E, C, S], _FP32, isOutput=False)
    gt = nc.declare_dram_parameter("gt", [C, E], _FP32, isOutput=False)
    out = nc.declare_dram_parameter("out", [C, S], _FP32, isOutput=True)

    n_ops = N_CTILES * E  # 32 expert-accumulate steps per rep
    n_blocks = reps * N_CTILES

    import contextlib

    with contextlib.ExitStack() as stack:
        lbuf = stack.enter_context(nc.sbuf_tensor([P, NBUF * S], _FP32))
        accbuf = stack.enter_context(nc.sbuf_tensor([P, 2 * S], _FP32))
        gbuf = stack.enter_context(nc.sbuf_tensor([P, N_CTILES * E], _FP32))
        ld_sems = [
            stack.enter_context(nc.semaphore(f"ld{j}")) for j in range(NBUF)
        ]  # per l-ring-slot load completion
        st_sems = [
            stack.enter_context(nc.semaphore(f"st{p}")) for p in range(2)
        ]  # per acc-parity store completion
        g_sem = stack.enter_context(nc.semaphore("g_sem"))
        v_sem = stack.enter_context(nc.semaphore("v_sem"))
        block = stack.enter_context(nc.Block())

        @block.sync
        def _(sync):
            for ci in range(N_CTILES):
                sync.dma_start(
                    out=gbuf[:, ci * E : (ci + 1) * E],
                    in_=gt[ci * P : (ci + 1) * P, :],
                ).then_inc(g_sem, 16)
            for og in range(reps * n_ops):
                ci, e = divmod(og % n_ops, E)
                slot = og % NBUF
                if og >= NBUF:
                    # slot reused: its previous occupant must be consumed
                    sync.wait_ge(v_sem, og - NBUF + 1)
                sync.dma_start(
                    out=lbuf[:, slot * S : (slot + 1) * S],
                    in_=l[e, ci * P : (ci + 1) * P, :],
                ).then_inc(ld_sems[slot], 16)

        @block.vector
        def _(vector):
            vector.wait_ge(g_sem, 16 * N_CTILES)
            for og in range(reps * n_ops):
                ci, e = divmod(og % n_ops, E)
                slot = og % NBUF
                sb = og // E  # global ci-block index
                acc = accbuf[:, (sb % 2) * S : (sb % 2 + 1) * S]
                lt = lbuf[:, slot * S : (slot + 1) * S]
                gcol = gbuf[:, ci * E + e : ci * E + e + 1]
                vector.wait_ge(ld_sems[slot], 16 * (og // NBUF + 1))
                if e == 0:
                    if sb >= 2:
                        # acc slot recycled: store of block sb-2 must be done
                        vector.wait_ge(st_sems[sb % 2], 16 * (sb // 2))
                    vector.tensor_scalar_mul(acc, lt, gcol).then_inc(v_sem, 1)
                else:
                    vector.scalar_tensor_tensor(
                        acc,
                        lt,
                        gcol,
                        acc,
                        op0=mybir.AluOpType.mult,
                        op1=mybir.AluOpType.add,
                    ).then_inc(v_sem, 1)

        @block.scalar
        def _(scalar):
            for sb in range(n_blocks):
                ci = sb % N_CTILES
                scalar.wait_ge(v_sem, E * (sb + 1))
                scalar.dma_start(
                    out=out[ci * P : (ci + 1) * P, :],
                    in_=accbuf[:, (sb % 2) * S : (sb % 2 + 1) * S],
                ).then_inc(st_sems[sb % 2], 16)
            scalar.wait_ge(st_sems[0], 16 * ((n_blocks + 1) // 2))
            scalar.wait_ge(st_sems[1], 16 * (n_blocks // 2))

    return nc


def _get_program() -> bass.Bass:
    global _program
    if _program is None:
        _program = _build_program()
    return _program


def _shard_inputs(l_learner: np.ndarray, g: np.ndarray) -> list[dict[str, np.ndarray]]:
    l_learner = np.asarray(l_learner, dtype=np.float32)
    g = np.asarray(g, dtype=np.float32)
    in_maps = []
    for b in range(BATCH):
        lb = np.ascontiguousarray(l_learner[:, b]).reshape(N_EXPERTS, CHANNELS, S)
        gb = np.ascontiguousarray(g[b].reshape(N_EXPERTS, CHANNELS).T)
        in_maps.append({"l": lb, "gt": gb})
    return in_maps


def kernel(l_learner: np.ndarray, g: np.ndarray) -> np.ndarray:
    nc = _get_program()
    in_maps = _shard_inputs(l_learner, g)
    res = run_bass_kernel_spmd(nc, in_maps, list(range(N_CORES)))
    return np.stack(
        [res.results[b]["out"].reshape(CHANNELS, H, W) for b in range(BATCH)], axis=0
    )
